# revision 15
# baseline (speedup 1.0000x reference)
"""KoLeo loss kernel for Trainium2 (8 NeuronCores) — fp8 DoubleRow edition.

loss = -mean_i log( || xn_i - xn_{nn(i)} ||_2 + eps ),  xn = row-normalized x,
nn(i) = argmax_{j != i} xn_i . xn_j.

For unit rows ||xn_i - xn_j||^2 = 2 - 2*sim_ij, so only the row MAX of the
similarity matrix (diagonal excluded) is needed.  The gram runs on RAW
fp8(e4m3)-cast inputs with fp8 DoubleRow matmuls (2 k-tiles/instruction at
0.5 cycles/row: 4x the bf16 matmul rate).  Numerics validated in numpy:
raw-e4m3 gram + fp8-derived norms + fp16 scale/max reproduces the fp32
loss to ~2e-4 (gate 2e-2).

Distribution: rows sharded 1024/core; each core gets the full x^T with its
columns ROTATED so its own rows sit at columns 0..1023 — SPMD-identical
program (static diagonal masking), only data differs.

The gram is computed TRANSPOSED vs the v1 kernel: out tile = [128 j rows
(partition), 512 own-i (free)], lhsT = j-tile, rhs = own rows.  This makes
the column scale 1/|x_j| a PER-PARTITION vector, so the PSUM evacuation
applies it for free (DVE tensor_scalar / ACT activation-scale), and the
per-own-row max accumulates across the 64 j-tiles with cheap fp16 2x
elementwise tensor_max — no per-tile reductions at all.  Norms come from
the gram itself: each j-tile's 128x128 fp8 self-product's diagonal is
Sum x8^2 (identity-mask multiply + free reduce), so there is no separate
squares pass over the fp32 stream.  Cost model: DMA floor 93 us for the
32 MB fp32 stream; ACT/DVE/Pool each balanced at ~90 us; PE ~63 us.

Per-core program:
  stage A (per 1024-col group, 8 groups): 8 [128,1024] fp32 DMAs (SP ring);
    fp8 casts split ACT/Pool/DVE into k-pair tiles [128, 2, 1024] (DoubleRow
    operand layout); per j-tile: fp8 self-matmul [128,128], ACT evac (f16),
    GpSimd identity-mask multiply, DVE free-reduce -> nsq column; one ACT
    Sqrt + DVE reciprocal per group -> invn [128, 64] (column jt = 1/|x_j|
    for j-tile jt, partition layout).
  stage B: per out tile (jt, h): 4 DoubleRow matmuls (PSUM f32, 6 banks);
    -(1+1e-3) identity-stripe multiply on the 8 self tiles; scale-evac
    (x invn[:, jt], f16) split ACT/DVE; running elementwise tensor_max into
    acc[h] on DVE (fp16 2x).
  stage C: 8 PE transposes of acc + DVE free reduce_max -> s per own row
    (partition layout); s *= invn[:, 0:8] (own rows ARE j-tiles 0..7);
    clamp; 0.5*ln(2-2s) via ACT Ln and a 0.5-weighted ones-matmul.
Host: loss = -(sum of 8 partials) / 8192.
"""

import os
import sys

import numpy as np

for _p in ("/opt/trn_rl_repo", "/root/.axon_site/_ro/trn_rl_repo"):
    if os.path.isdir(_p) and _p not in sys.path:
        sys.path.insert(0, _p)

import ml_dtypes  # noqa: E402
from contextlib import ExitStack  # noqa: E402

import concourse.bass as bass  # noqa: E402
import concourse.tile as tile  # noqa: E402
from concourse import bacc, mybir  # noqa: E402
from concourse.bass_utils import run_bass_kernel_spmd  # noqa: E402

N = 8192          # rows
D = 1024          # features
NCORES = 8
R = N // NCORES   # rows per core (1024)
KT = D // 128     # 8 k-tiles
KP = KT // 2      # 4 k-pairs (DoubleRow)
GW = 1024         # group width (8 j-tiles)
NG = N // GW      # 8 groups
NJ = N // 128     # 64 j-tiles
NH = R // 512     # 2 own-row halves
EPS = 1e-8

F32 = mybir.dt.float32
F16 = mybir.dt.float16
BF16 = mybir.dt.bfloat16
FP8 = mybir.dt.float8e4
AF = mybir.ActivationFunctionType
AX = mybir.AxisListType
PM = mybir.MatmulPerfMode

# ---- engine split knobs (tuned against the cost-model timeline) ----
CAST_ACT = 18     # of 64 [128,1024] casts on ACT
CAST_DVE = 2      # on DVE (rest GpSimd)
EVAC_ACT = 85     # of 128 scale-evacs on ACT (rest DVE)

_CACHE = {}


def _build_program():
    nc = bacc.Bacc("TRN2", target_bir_lowering=False, debug=False,
                   num_devices=NCORES)

    xt = nc.dram_tensor("xt", [D, N], F32, kind="ExternalInput").ap()
    losspart = nc.dram_tensor("losspart", [1, 1], F32, kind="ExternalOutput").ap()

    negid_np = np.ones((128, 128), np.float32)
    np.fill_diagonal(negid_np, -(1.0 + 1e-3))
    negid_d = nc.inline_tensor(negid_np, "negid")
    identf_np = np.zeros((128, 128), np.float16)
    np.fill_diagonal(identf_np, 1.0)
    identf_d = nc.inline_tensor(identf_np, "identf")
    half_col_d = nc.inline_tensor(np.full((128, 1), 0.5, np.float32), "half_col")
    two_col_d = nc.inline_tensor(np.full((128, 1), 2.0, np.float32), "two_col")
    ident_d = nc.inline_tensor(np.eye(128, dtype=np.float32), "ident")

    with tile.TileContext(nc) as tc, ExitStack() as ctx:
        const_pool = ctx.enter_context(tc.tile_pool(name="const", bufs=1))
        stg_pool = ctx.enter_context(tc.tile_pool(name="stg", bufs=10))
        x8_pool = ctx.enter_context(tc.tile_pool(name="x8", bufs=1))
        dg_pool = ctx.enter_context(tc.tile_pool(name="dg", bufs=4))
        ttr_pool = ctx.enter_context(tc.tile_pool(name="ttr", bufs=10))
        stat_pool = ctx.enter_context(tc.tile_pool(name="stat", bufs=1))
        ps_d = ctx.enter_context(tc.tile_pool(name="psD", bufs=2, space="PSUM"))
        ps_s = ctx.enter_context(tc.tile_pool(name="psS", bufs=5, space="PSUM"))

        # preload ACT tables while idle
        pre = stat_pool.tile([128, 3], F32, tag="pre")
        nc.vector.memset(pre[:], 1.0)
        nc.scalar.activation(pre[:, 2:3], pre[:, 2:3], AF.Ln)
        nc.scalar.activation(pre[:, 1:2], pre[:, 1:2], AF.Sqrt)
        nc.scalar.activation(pre[:, 0:1], pre[:, 0:1], AF.Square)

        negid = const_pool.tile([128, 128], F32, tag="negid")
        nc.gpsimd.dma_start(negid[:], negid_d[:, :])
        identf = const_pool.tile([128, 128], F16, tag="identf")
        nc.gpsimd.dma_start(identf[:], identf_d[:, :])
        half_col = const_pool.tile([128, 1], F32, tag="half_col")
        nc.gpsimd.dma_start(half_col[:], half_col_d[:, :])
        two_col = const_pool.tile([128, 1], F32, tag="two_col")
        nc.gpsimd.dma_start(two_col[:], two_col_d[:, :])
        ident = const_pool.tile([128, 128], F32, tag="ident")
        nc.gpsimd.dma_start(ident[:], ident_d[:, :])

        nsq = stat_pool.tile([128, NJ], F32, tag="nsq")
        invn = stat_pool.tile([128, NJ], F32, tag="invn")
        NQ = 4    # parallel max-accumulator chains per half
        acc = [[stat_pool.tile([128, 512], F16, tag=f"acc{h}_{q}",
                               name=f"acc{h}_{q}")
                for q in range(NQ)] for h in range(NH)]
        sbuf_s = stat_pool.tile([128, 8], F32, tag="sbuf_s")
        logbuf = stat_pool.tile([128, 8], F32, tag="logbuf")

        # fp8 k-pair operand tiles: xp[p][g] is [128, 2, GW]
        xp = [[x8_pool.tile([128, 2, GW], FP8, tag=f"xp{p}_{g}", name=f"xp{p}_{g}")
               for g in range(NG)] for p in range(KP)]

        def prop_pick(i, total, quota_a, quota_b):
            """3-way proportional schedule: 'a' quota_a times, 'b' quota_b."""
            if (i * quota_a) // total != ((i + 1) * quota_a) // total:
                return "a"
            if ((i * (quota_a + quota_b)) // total
                    != ((i + 1) * (quota_a + quota_b)) // total):
                return "b"
            return "c"

        # ---- per group: load + casts + norms (A) then gram tiles (B) ----
        # (single loop keeps PE program order pipelined with the DMA stream;
        #  DMAs are [128, 2*GW] superloads spanning 2 groups to halve the
        #  SP-sequencer issue cost, which otherwise caps the stream)
        ev_i = 0

        def gram_block(g):
            nonlocal ev_i
            for jj in range(8):
                jt = g * 8 + jj
                off = jj * 128
                for h in range(NH):
                    s_ps = ps_s.tile([128, 512], F32, name="s_ps")
                    for p in range(KP):
                        lhsT = xp[p][g][:, :, off:off + 128]
                        rhs = xp[p][0][:, :, h * 512:(h + 1) * 512]
                        nc.tensor.matmul(s_ps[:], lhsT, rhs,
                                         start=(p == 0), stop=(p == KP - 1),
                                         perf_mode=PM.DoubleRow)
                    if jt < 8 and h == jt // 4:
                        o2 = (jt % 4) * 128
                        nc.vector.tensor_mul(s_ps[:, o2:o2 + 128],
                                             s_ps[:, o2:o2 + 128], negid[:])
                    ttr = ttr_pool.tile([128, 512], F16, tag="ttr", name="ttr")
                    if (ev_i * EVAC_ACT) // 128 != ((ev_i + 1) * EVAC_ACT) // 128:
                        nc.scalar.activation(ttr[:], s_ps[:], AF.Copy,
                                             scale=invn[:, jt:jt + 1])
                    else:
                        nc.vector.tensor_scalar_mul(ttr[:], s_ps[:],
                                                    invn[:, jt:jt + 1])
                    ev_i += 1
                    q = jj % 4
                    if jt < 8:
                        nc.vector.tensor_copy(acc[h][q][:], ttr[:])
                    else:
                        nc.vector.tensor_max(acc[h][q][:], acc[h][q][:],
                                             ttr[:])

        wide = [None] * KT
        for g in range(NG):
            for k in range(KT):
                if g % 2 == 0:
                    tw = stg_pool.tile([128, 2 * GW], F32, tag="stg", name="tw")
                    nc.sync.dma_start(tw[:], xt[k * 128:(k + 1) * 128,
                                                g * GW:(g + 2) * GW])
                    wide[k] = tw
                t = wide[k][:, (g % 2) * GW:(g % 2 + 1) * GW]
                dst = xp[k // 2][g][:, k % 2, :]
                eng = prop_pick(g * KT + k, NG * KT, CAST_ACT, CAST_DVE)
                if eng == "a":
                    nc.scalar.copy(dst, t)
                elif eng == "b":
                    nc.vector.tensor_copy(dst, t)
                else:
                    nc.gpsimd.tensor_copy(dst, t)
            # self-products: per j-tile in this group, diagonal -> norms^2
            for jj in range(8):
                jt = g * 8 + jj
                off = jj * 128
                sp = ps_d.tile([128, 128], F32, tag="sp", name="sp")
                for p in range(KP):
                    sl = xp[p][g][:, :, off:off + 128]
                    nc.tensor.matmul(sp[:], sl, sl,
                                     start=(p == 0), stop=(p == KP - 1),
                                     perf_mode=PM.DoubleRow)
                dg = dg_pool.tile([128, 128], F16, tag="dg", name="dg")
                nc.scalar.copy(dg[:], sp[:])
                dgm = dg_pool.tile([128, 128], F16, tag="dgm", name="dgm")
                nc.gpsimd.tensor_mul(dgm[:], dg[:], identf[:])
                nc.vector.tensor_reduce(nsq[:, jt:jt + 1], dgm[:], axis=AX.X,
                                        op=mybir.AluOpType.add)
            # 1/|x_j| for this group's 8 j-tiles (partition layout)
            gsl = slice(g * 8, g * 8 + 8)
            nrm8 = dg_pool.tile([128, 8], F32, tag="nrm8", name="nrm8")
            nc.scalar.activation(nrm8[:], nsq[:, gsl], AF.Sqrt)
            nc.vector.reciprocal(invn[:, gsl], nrm8[:])

            # one-group software pipeline: this group's norm chain runs
            # while the PREVIOUS group's gram bulk keeps the engines fed
            if g >= 1:
                gram_block(g - 1)
        gram_block(NG - 1)

        # ---- stage C: merge chains + partition max via transpose ----
        for h in range(NH):
            nc.vector.tensor_max(acc[h][0][:], acc[h][0][:], acc[h][1][:])
            nc.vector.tensor_max(acc[h][2][:], acc[h][2][:], acc[h][3][:])
            nc.vector.tensor_max(acc[h][0][:], acc[h][0][:], acc[h][2][:])
            for tq in range(4):
                m = h * 4 + tq
                tp = ps_d.tile([128, 128], F16, tag="tp", name="tp", bufs=1)
                nc.tensor.transpose(tp[:],
                                    acc[h][0][:, tq * 128:(tq + 1) * 128],
                                    identf[:])
                nc.vector.reduce_max(sbuf_s[:, m:m + 1], tp[:], axis=AX.X)
        nc.vector.tensor_mul(sbuf_s[:], sbuf_s[:], invn[:, 0:8])
        nc.vector.tensor_scalar_min(sbuf_s[:], sbuf_s[:], 1.0 - 1e-7)
        nc.scalar.activation(logbuf[:], sbuf_s[:], AF.Ln,
                             bias=two_col[:], scale=-2.0)

        fin_full = ps_d.tile([1, 128], F32, tag="sp")
        fin = fin_full[:, :8]
        nc.tensor.matmul(fin[:], half_col[:], logbuf[:], start=True, stop=True)
        tot = stat_pool.tile([1, 1], F32, tag="tot")
        nc.vector.reduce_sum(tot[:], fin[:], axis=AX.X)
        nc.sync.dma_start(losspart[:], tot[:])

    nc.compile()
    return nc


def _run(student_output: np.ndarray, **spmd_kwargs):
    x = np.asarray(student_output, dtype=np.float32)
    assert x.shape == (N, D), x.shape

    if "nc" not in _CACHE:
        _CACHE["nc"] = _build_program()
    nc = _CACHE["nc"]

    xtf = np.ascontiguousarray(x.T)  # [D, N]
    in_maps = []
    for c in range(NCORES):
        s = c * R
        rolled = np.concatenate([xtf[:, s:], xtf[:, :s]], axis=1) if s else xtf
        in_maps.append({"xt": np.ascontiguousarray(rolled)})

    res = None
    for attempt in range(3):
        try:
            res = run_bass_kernel_spmd(nc, in_maps, list(range(NCORES)),
                                       **spmd_kwargs)
            break
        except Exception:
            # transient NRT_EXEC_UNIT_UNRECOVERABLE: retry with fresh backends
            if attempt == 2:
                raise
            import time

            try:
                import jax

                jax.clear_caches()
                jax.extend.backend.clear_backends()
            except Exception:
                pass
            time.sleep(5.0)
    total = np.float64(0.0)
    for c in range(NCORES):
        total += np.float64(res.results[c]["losspart"][0, 0])
    return np.asarray(-total / N, dtype=np.float32), res


def kernel(student_output: np.ndarray) -> np.ndarray:
    return _run(student_output)[0]
